# revision 36
# baseline (speedup 1.0000x reference)
"""Distributed Trainium2 (8 NeuronCores) kernel for nn_AdaptiveAttention.

Reference computation (b=2, n=2048, d=1024, 16 heads x 64):
    qkv = x @ W_qkv; q,k,v = split(qkv)
    attn = softmax(mask(q k^T / sqrt(dh)))
    out  = (attn @ v) @ W_out + b_out

Sharding: core c in [0,8) handles batch b = c//4 and head group g = c%4
(heads 4g..4g+3).  Data parallel over b, tensor parallel over heads.

Design:
 - Projections run upfront, pipelined with the input DMA.  A tiny
   dummy AllGather during this phase warms up the collectives
   firmware (the first collective otherwise pays a ~30us cold start).
 - Attention runs as 4 merged rounds, one per (head-pair, i-half).
   The two heads of a pair occupy SBUF partitions 0:64 / 64:128 and
   their q.kT score matmuls issue as paired 64x128 row tiles of the
   PE array (tile_position).
 - Softmax elementwise work is spread across engines by jc parity:
     even jc: ACT exps both heads; DVE multiplies both by the 0/1
              bf16 mask (2x mode).
     odd jc:  ACT exps head A, DVE does A's mask-mul; head B runs a
              masked Schraudolph exp: DVE adds a {0,-80} fp8 mask
              offset to the PSUM scores (masked scores land at
              ~exp(-80) = 0 in bf16), then GpSimd's tensor_scalar
              computes int16(a*x+b) whose bitcast IS bf16 exp(x)
              (~3% elementwise error on half the heads, washing to
              ~1% after softmax normalization and head mixing).
 - attn@v for each jc is emitted 2-3 slots late (software pipelining)
   so the PE FIFO never blocks the next jc's scores on the current
   jc's exp/mask chain.
 - attn@v uses a 65th ones-column in v so the softmax denominators
   fall out of the accumulation for free.
 - Round tails only copy the PSUM accumulator out and DMA it to the
   AllGather buffer; normalization happens after the gather, where the
   denominators sit 8-per-partition (fast reciprocal shape), via a
   tiny expansion matmul (contract 8) - and is deferred past the
   rounds so PSUM stays free; it overlaps the last AG's flight
   together with the first half of the output projection.
 - Mask halves are double-buffered in SBUF (i-half 0 for rounds 1-2,
   then the same SBUF is refilled with i-half 1 for rounds 3-4, write
   pipelining automatically one jc behind the last reader).
"""

import numpy as np
import ml_dtypes

import concourse.bass as bass
import concourse.tile as tile
from concourse import bacc, mybir
from concourse import bass_utils

BF16 = ml_dtypes.bfloat16
FP8 = ml_dtypes.float8_e4m3

B = 2
N = 2048
D = 1024
HEADS = 16
HD = 64
SCALE = HD ** -0.5
N_CORES = 8
HPC = 4
IB = 1024
NJ = N // 128  # 16 j-chunks

# bf16-domain Schraudolph: exp(x) ~ bitcast_bf16(int16(x*A16 + B16))
A16 = float(np.float32(2 ** 7 / np.log(2)))
B16 = float(np.float32(127 * 2 ** 7 - 366250 / 2 ** 16))
MOFF = -80.0  # masked-score offset: exp(s-80) == 0 in bf16

# rounds r -> (pair, ib2); ib2=0 chunks complete first so the
# out-projection of the first i-half can overlap the last AG
ROUNDS = [(0, 0), (1, 0), (0, 1), (1, 1)]

_cached_nc = None
_last_in_maps = None
_last_res = None


def _build():
    nc = bacc.Bacc("TRN2", target_bir_lowering=False, debug=False,
                   num_devices=N_CORES)

    f32 = mybir.dt.float32
    bf = mybir.dt.bfloat16
    i16 = mybir.dt.int16
    f8 = mybir.dt.float8e4

    xt = nc.dram_tensor("xt", [D, N], bf, kind="ExternalInput")
    wqkv = nc.dram_tensor("wqkv", [D, 768], bf, kind="ExternalInput")
    m01t = nc.dram_tensor("m01t", [N, N], bf, kind="ExternalInput")
    mofft = nc.dram_tensor("mofft", [N, N], f8, kind="ExternalInput")
    wout = nc.dram_tensor("wout", [D, D], bf, kind="ExternalInput")
    emat = nc.dram_tensor("emat", [8, 512], bf, kind="ExternalInput")
    out = nc.dram_tensor("out", [N // 4, D], bf, kind="ExternalOutput")

    with tile.TileContext(nc) as tc:
        with (
            tc.tile_pool(name="res", bufs=1) as res,
            tc.tile_pool(name="dram", bufs=1, space="DRAM") as dram,
            tc.tile_pool(name="pp", bufs=5) as pool_p,
            tc.tile_pool(name="pe", bufs=4) as pool_e,
            tc.tile_pool(name="pao", bufs=3) as pao,
            tc.tile_pool(name="ps_pool", bufs=2, space="PSUM") as ps_pool,
            tc.tile_pool(name="pacc", bufs=2, space="PSUM") as pacc,
        ):
            # ---------------- resident tensors ----------------
            qkt = res.tile([128, 4 * N], bf)      # [qT01|qT23|kT01|kT23]
            v_aug = res.tile([128, NJ * 260], bf)  # per jc: 4x(64 v + ones)
            m01 = res.tile([128, NJ * IB], bf)     # 0/1 mask, current i-half
            moff = res.tile([128, NJ * IB], f8)    # {0,-80} mask, same half
            wout_sb = res.tile([128, 8 * D], bf)
            emat_sb = res.tile([8, 512], bf, name="emat_sb")
            att_sb = [res.tile([128, 4 * 256], bf, name=f"att{r}")
                      for r in range(4)]
            attn_n = [res.tile([128, 4 * 256], bf, name=f"attn{r}")
                      for r in range(4)]
            sums_sb = [res.tile([8, 256], bf, name=f"sums{r}")
                       for r in range(4)]

            ag_ins = [dram.tile([130, IB], bf, name=f"ag_in{r}")
                      for r in range(4)]
            ag_outs = [dram.tile([8 * 130, IB], bf, name=f"ag_out{r}",
                                 addr_space="Shared")
                       for r in range(4)]

            nc.vector.memset(v_aug[:], 1.0)

            pid = nc.sync.partition_id()
            i0c = (pid % 4) * 256    # my 256-col slice within a chunk
            goff = (pid // 4) * 520  # my batch group's rank-block offset

            def load_mask_half(ib2, q):
                for jc in range(NJ):
                    q.dma_start(
                        m01[:, IB * jc:IB * (jc + 1)],
                        m01t[128 * jc:128 * (jc + 1),
                             IB * ib2:IB * (ib2 + 1)])
                    q.dma_start(
                        moff[:, IB * jc:IB * (jc + 1)],
                        mofft[128 * jc:128 * (jc + 1),
                              IB * ib2:IB * (ib2 + 1)])

            with tc.tile_pool(name="ph0", bufs=1) as p0:
                xtr = p0.tile([128, 8 * N], bf)
                wr = p0.tile([128, 8 * 768], bf)

                # wqkv on the gpsimd queue, x on the sync queue so the
                # two streams flow in parallel and the first projection
                # matmul is gated by ~1 chunk, not the serialized pair
                for k in range(8):
                    nc.gpsimd.dma_start(wr[:, 768 * k:768 * (k + 1)],
                                        wqkv[128 * k:128 * (k + 1), :])
                    nc.sync.dma_start(xtr[:, N * k:N * (k + 1)],
                                      xt[128 * k:128 * (k + 1), :])
                nc.gpsimd.dma_start(emat_sb[:], emat[:])

                # warm-up burst: dummy matmuls during the DMA ramp flip
                # the PE's HAM clock gate to 8/8 before real work issues
                zw = p0.tile([128, 256], bf, name="zw")
                nc.vector.memset(zw[:], 0.0)
                psw = pacc.tile([64, 256], f32, name="psw", tag="acc")
                for _ in range(30):
                    nc.tensor.matmul(psw[:], zw[:, 0:64], zw[:],
                                     start=True, stop=True)
                load_mask_half(0, nc.gpsimd)
                for k in range(8):
                    nc.gpsimd.dma_start(wout_sb[:, D * k:D * (k + 1)],
                                        wout[128 * k:128 * (k + 1), :])

                # tiny dummy AllGather to warm up the collectives
                # firmware before the first real one (~30us cold start)
                agw_in = dram.tile([8, 16], bf, name="agw_in")
                agw_out = dram.tile([64, 16], bf, name="agw_out",
                                    addr_space="Shared")
                warm_sb = res.tile([8, 16], bf, name="warm_sb")
                nc.vector.memset(warm_sb[:], 0.0)
                nc.sync.dma_start(agw_in[:], warm_sb[:])
                nc.gpsimd.collective_compute(
                    "AllGather",
                    mybir.AluOpType.bypass,
                    replica_groups=[[0, 1, 2, 3, 4, 5, 6, 7]],
                    ins=[agw_in[:].opt()],
                    outs=[agw_out[:].opt()],
                )

                # ---------------- projections (upfront) ----------------
                def proj_qk_group(t_i, nb):
                    wcol = 128 * t_i
                    ps = ps_pool.tile([128, 512], f32, name="ps_qk",
                                      tag="mm")
                    for k in range(8):
                        nc.tensor.matmul(
                            ps[:],
                            wr[:, 768 * k + wcol:768 * k + wcol + 128],
                            xtr[:, N * k + 512 * nb:N * k + 512 * nb + 512],
                            start=(k == 0), stop=(k == 7),
                        )
                    nc.scalar.copy(
                        qkt[:, N * t_i + 512 * nb:N * t_i + 512 * nb + 512],
                        ps[:])

                def proj_v_group(jc):
                    ps = ps_pool.tile([128, 256], f32, name="ps_v", tag="mm")
                    for k in range(8):
                        nc.tensor.matmul(
                            ps[:],
                            xtr[:, N * k + 128 * jc:N * k + 128 * jc + 128],
                            wr[:, 768 * k + 512:768 * k + 768],
                            start=(k == 0), stop=(k == 7),
                        )
                    for h in range(4):
                        nc.vector.tensor_copy(
                            v_aug[:, 260 * jc + 65 * h:260 * jc + 65 * h + 64],
                            ps[:, 64 * h:64 * h + 64])

                for nb in range(4):
                    proj_qk_group(0, nb)
                for nb in range(4):
                    proj_qk_group(2, nb)
                for jc in range(NJ):
                    proj_v_group(jc)
                for nb in range(4):
                    proj_qk_group(1, nb)
                for nb in range(4):
                    proj_qk_group(3, nb)

            # ---------------- merged attention rounds ----------------
            def do_round(r):
                pair, ib2 = ROUNDS[r]
                q_off = N * pair + IB * ib2
                k_off = N * (2 + pair)
                lA, lB = 2 * pair, 2 * pair + 1

                accA = pacc.tile([65, IB], f32, name="accA", tag="acc")
                accB = pacc.tile([65, IB], f32, name="accB", tag="acc")

                def emit_av(jc, pA, pB, bcast):
                    for ih in range(2):
                        nc.tensor.matmul(
                            accA[:, 512 * ih:512 * ih + 512],
                            v_aug[:, 260 * jc + 65 * lA:
                                  260 * jc + 65 * lA + 65],
                            pA[:, 512 * ih:512 * ih + 512],
                            start=(jc == 0), stop=(jc == NJ - 1),
                        )
                    for ih in range(2):
                        rhsB = pB[:, 512 * ih:512 * ih + 512]
                        if bcast:
                            rhsB = rhsB.bitcast(bf)
                        nc.tensor.matmul(
                            accB[:, 512 * ih:512 * ih + 512],
                            v_aug[:, 260 * jc + 65 * lB:
                                  260 * jc + 65 * lB + 65],
                            rhsB,
                            start=(jc == 0), stop=(jc == NJ - 1),
                        )

                # software pipelining: attn@v for jc is emitted 2 slots
                # late so the PE FIFO never blocks scores(jc+1) on jc's
                # exp/mask chain
                pq = []
                for jc in range(NJ):
                    sA = ps_pool.tile([128, IB], f32, name="sA", tag="mm")
                    sB = ps_pool.tile([128, IB], f32, name="sB", tag="mm")
                    for ih in range(2):
                        nc.tensor.matmul(
                            sA[:, 512 * ih:512 * ih + 512],
                            qkt[0:64, k_off + 128 * jc:k_off + 128 * jc + 128],
                            qkt[0:64, q_off + 512 * ih:q_off + 512 * ih + 512],
                            start=True, stop=True, tile_position=(0, 0),
                        )
                        nc.tensor.matmul(
                            sB[:, 512 * ih:512 * ih + 512],
                            qkt[64:128,
                                k_off + 128 * jc:k_off + 128 * jc + 128],
                            qkt[64:128,
                                q_off + 512 * ih:q_off + 512 * ih + 512],
                            start=True, stop=True, tile_position=(64, 0),
                        )
                    mrow = m01[:, IB * jc:IB * jc + IB]
                    morow = moff[:, IB * jc:IB * jc + IB]
                    # head A always: table exp on ACT + 0/1 mask-mul DVE
                    eA = pool_e.tile([128, IB], bf, name="eA", tag="eA")
                    nc.scalar.activation(
                        eA[:], sA[:], mybir.ActivationFunctionType.Exp)
                    if jc % 2 == 0:
                        # head B via ACT exp too; DVE multiplies both
                        eB = pool_e.tile([128, IB], bf, name="eB", tag="eB")
                        nc.scalar.activation(
                            eB[:], sB[:], mybir.ActivationFunctionType.Exp)
                        pA = pool_p.tile([128, IB], bf, name="pA", tag="pA")
                        nc.vector.tensor_mul(pA[:], eA[:], mrow)
                        pB = pool_p.tile([128, IB], bf, name="pB", tag="pB")
                        nc.vector.tensor_mul(pB[:], eB[:], mrow)
                        bcast = False
                    else:
                        # head B: masked Schraudolph - DVE folds the
                        # {0,-80} offset in (ready right after scores,
                        # so emitted before the ACT-gated mulA), GpSimd
                        # does the int16 fma convert
                        aB = pool_e.tile([128, IB], bf, name="aB", tag="aB")
                        nc.vector.tensor_tensor(
                            aB[:], sB[:], morow, mybir.AluOpType.add)
                        pA = pool_p.tile([128, IB], bf, name="pA", tag="pA")
                        nc.vector.tensor_mul(pA[:], eA[:], mrow)
                        pB = pool_p.tile([128, IB], i16, name="tB", tag="pB")
                        nc.gpsimd.tensor_scalar(
                            pB[:], aB[:], A16, B16,
                            mybir.AluOpType.mult, mybir.AluOpType.add)
                        bcast = True
                    pq.append((jc, pA, pB, bcast))
                    if len(pq) > 3:
                        emit_av(*pq.pop(0))
                while pq:
                    emit_av(*pq.pop(0))

                # round tail: evacuate + ship (no normalize here)
                for hh, acc in ((0, accA), (1, accB)):
                    ao = pao.tile([65, IB], bf, name="ao", tag="ao")
                    nc.vector.tensor_copy(ao[:], acc[:])
                    nc.sync.dma_start(
                        ag_ins[r][64 * hh:64 * hh + 64, :], ao[0:64, :])
                    nc.sync.dma_start(
                        ag_ins[r][128 + hh:129 + hh, :], ao[64:65, :])
                nc.gpsimd.collective_compute(
                    "AllGather",
                    mybir.AluOpType.bypass,
                    replica_groups=[[0, 1, 2, 3, 4, 5, 6, 7]],
                    ins=[ag_ins[r][:].opt()],
                    outs=[ag_outs[r][:].opt()],
                )

            def emit_att_reads(r):
                for gp in range(4):
                    nc.sync.dma_start(
                        att_sb[r][:, 256 * gp:256 * gp + 256],
                        ag_outs[r][bass.ds(goff + 130 * gp, 128),
                                   bass.ds(i0c, 256)])
                    nc.sync.dma_start(
                        sums_sb[r][2 * gp:2 * gp + 2, :],
                        ag_outs[r][bass.ds(goff + 130 * gp + 128, 2),
                                   bass.ds(i0c, 256)])

            def normalize(r):
                rec = res.tile([8, 256], bf, name=f"rec{r}")
                with nc.allow_low_precision(reason="softmax recip"):
                    nc.vector.reciprocal(rec[:], sums_sb[r][:])
                for gp in range(4):
                    bc = ps_pool.tile([128, 256], f32, name="bc", tag="mm")
                    nc.tensor.matmul(
                        bc[:], emat_sb[:, 128 * gp:128 * gp + 128], rec[:],
                        start=True, stop=True)
                    with nc.allow_low_precision(reason="softmax norm"):
                        nc.vector.tensor_mul(
                            attn_n[r][:, 256 * gp:256 * gp + 256],
                            att_sb[r][:, 256 * gp:256 * gp + 256],
                            bc[:])

            with tc.tile_pool(name="ph2", bufs=1) as p2:
                do_round(0)
                do_round(1)
                load_mask_half(1, nc.sync)  # refill mask for rounds 3-4
                emit_att_reads(0)   # AG0 fired a round ago
                do_round(2)
                emit_att_reads(1)
                emit_att_reads(2)   # AG2 completes mid-round-4
                do_round(3)

                # ---------------- normalize + output projection ----------------
                def out_proj(ib2):
                    ra = 2 * ib2
                    rb = 2 * ib2 + 1
                    for mo in range(2):
                        for nh in range(2):
                            ps = ps_pool.tile([128, 512], f32, name="ps_o",
                                              tag="mm")
                            ki = 0
                            for gp in range(4):
                                for p, rr in ((0, ra), (1, rb)):
                                    nc.tensor.matmul(
                                        ps[:],
                                        attn_n[rr][:, 256 * gp + 128 * mo:
                                                   256 * gp + 128 * mo + 128],
                                        wout_sb[:, D * (2 * gp + p) + 512 * nh:
                                                D * (2 * gp + p) + 512 * nh + 512],
                                        start=(ki == 0), stop=(ki == 7),
                                    )
                                    ki += 1
                            ot = pao.tile([128, 512], bf, name="ot", tag="ot")
                            nc.scalar.copy(ot[:], ps[:])
                            nc.sync.dma_start(
                                out[256 * ib2 + 128 * mo:
                                    256 * ib2 + 128 * mo + 128,
                                    512 * nh:512 * nh + 512],
                                ot[:])

                normalize(0)
                normalize(1)
                normalize(2)
                out_proj(0)          # overlaps AG3 flight
                emit_att_reads(3)    # waits AG3; nothing queued behind
                normalize(3)
                out_proj(1)

    nc.compile()
    return nc


def _get_nc():
    global _cached_nc
    if _cached_nc is None:
        _cached_nc = _build()
    return _cached_nc


def kernel(x, mask, W_qkv, W_out, b_out):
    x = np.asarray(x, dtype=np.float32)
    mask = np.asarray(mask)
    W_qkv = np.asarray(W_qkv, dtype=np.float32)
    W_out = np.asarray(W_out, dtype=np.float32)
    b_out = np.asarray(b_out, dtype=np.float32)

    nc = _get_nc()

    mt = mask.reshape(N, N).T.astype(np.float32)
    m01_bf = np.ascontiguousarray(mt).astype(BF16)
    moff_f8 = np.ascontiguousarray((mt - 1.0) * (-MOFF)).astype(FP8)
    wout_bf = W_out.astype(BF16)
    # expansion matrix: emat[s, 128g+row] = 1 iff s == 2g + row//64
    em = np.zeros((8, 512), dtype=np.float32)
    for g in range(4):
        for s2 in range(2):
            em[2 * g + s2, 128 * g + 64 * s2:128 * g + 64 * s2 + 64] = 1.0
    em_bf = em.astype(BF16)

    in_maps = []
    for c in range(N_CORES):
        b = c // 4
        g = c % 4
        hs = slice(g * HPC * HD, (g + 1) * HPC * HD)
        wq = W_qkv[:, 0 * D:1 * D][:, hs] * np.float32(SCALE)
        wk = W_qkv[:, 1 * D:2 * D][:, hs]
        wv = W_qkv[:, 2 * D:3 * D][:, hs]
        wqkv_c = np.ascontiguousarray(
            np.concatenate([wq, wk, wv], axis=1)).astype(BF16)
        xt_c = np.ascontiguousarray(x[b].T).astype(BF16)
        in_maps.append({
            "xt": xt_c,
            "wqkv": wqkv_c,
            "m01t": m01_bf,
            "mofft": moff_f8,
            "wout": wout_bf,
            "emat": em_bf,
        })

    global _last_in_maps, _last_res
    _last_in_maps = in_maps

    res = bass_utils.run_bass_kernel_spmd(
        nc, in_maps, core_ids=list(range(N_CORES)))
    _last_res = res

    out_full = np.empty((B, N, D), dtype=np.float32)
    for c in range(N_CORES):
        b = c // 4
        g = c % 4
        core_out = res.results[c]["out"].astype(np.float32)
        out_full[b, 256 * g:256 * g + 256, :] = core_out[0:256]
        out_full[b, 1024 + 256 * g:1024 + 256 * g + 256, :] = core_out[256:512]
    out_full += b_out
    return out_full


# revision 37
# speedup vs baseline: 1.1558x; 1.1558x over previous
"""Distributed Trainium2 (8 NeuronCores) kernel for nn_AdaptiveAttention.

Reference computation (b=2, n=2048, d=1024, 16 heads x 64):
    qkv = x @ W_qkv; q,k,v = split(qkv)
    attn = softmax(mask(q k^T / sqrt(dh)))
    out  = (attn @ v) @ W_out + b_out

Sharding: core c in [0,8) handles batch b = c//4 and head group g = c%4
(heads 4g..4g+3).  Data parallel over b, tensor parallel over heads.

Design:
 - Projections run upfront, pipelined with the input DMA.  A tiny
   dummy AllGather during this phase warms up the collectives
   firmware (the first collective otherwise pays a ~30us cold start).
 - Attention runs as 4 merged rounds, one per (head-pair, i-half).
   The two heads of a pair occupy SBUF partitions 0:64 / 64:128 and
   their q.kT score matmuls issue as paired 64x128 row tiles of the
   PE array (tile_position).
 - Softmax elementwise work is spread across engines by jc parity:
     even jc: ACT exps both heads; DVE multiplies both by the 0/1
              bf16 mask (2x mode).
     odd jc:  ACT exps head A, DVE does A's mask-mul; head B runs a
              masked Schraudolph exp: DVE adds a {0,-80} fp8 mask
              offset to the PSUM scores (masked scores land at
              ~exp(-80) = 0 in bf16), then GpSimd's tensor_scalar
              computes int16(a*x+b) whose bitcast IS bf16 exp(x)
              (~3% elementwise error on half the heads, washing to
              ~1% after softmax normalization and head mixing).
 - attn@v for each jc is emitted 2-3 slots late (software pipelining)
   so the PE FIFO never blocks the next jc's scores on the current
   jc's exp/mask chain.
 - attn@v uses a 65th ones-column in v so the softmax denominators
   fall out of the accumulation for free.
 - Round tails only copy the PSUM accumulator out and DMA it to the
   AllGather buffer; normalization happens after the gather, where the
   denominators sit 8-per-partition (fast reciprocal shape), via a
   tiny expansion matmul (contract 8) - and is deferred past the
   rounds so PSUM stays free; it overlaps the last AG's flight
   together with the first half of the output projection.
 - Mask halves are double-buffered in SBUF (i-half 0 for rounds 1-2,
   then the same SBUF is refilled with i-half 1 for rounds 3-4, write
   pipelining automatically one jc behind the last reader).
"""

import numpy as np
import ml_dtypes

import concourse.bass as bass
import concourse.tile as tile
from concourse import bacc, mybir
from concourse import bass_utils

BF16 = ml_dtypes.bfloat16
FP8 = ml_dtypes.float8_e4m3

B = 2
N = 2048
D = 1024
HEADS = 16
HD = 64
SCALE = HD ** -0.5
N_CORES = 8
HPC = 4
IB = 1024
NJ = N // 128  # 16 j-chunks

# bf16-domain Schraudolph: exp(x) ~ bitcast_bf16(int16(x*A16 + B16))
A16 = float(np.float32(2 ** 7 / np.log(2)))
B16 = float(np.float32(127 * 2 ** 7 - 366250 / 2 ** 16))
MOFF = -80.0  # masked-score offset: exp(s-80) == 0 in bf16

# rounds r -> (pair, ib2); ib2=0 chunks complete first so the
# out-projection of the first i-half can overlap the last AG
ROUNDS = [(0, 0), (1, 0), (0, 1), (1, 1)]

_cached_nc = None
_last_in_maps = None
_last_res = None


def _build():
    nc = bacc.Bacc("TRN2", target_bir_lowering=False, debug=False,
                   num_devices=N_CORES)

    f32 = mybir.dt.float32
    bf = mybir.dt.bfloat16
    i16 = mybir.dt.int16
    f8 = mybir.dt.float8e4

    xt = nc.dram_tensor("xt", [D, N], bf, kind="ExternalInput")
    wqkv = nc.dram_tensor("wqkv", [D, 768], bf, kind="ExternalInput")
    m01t = nc.dram_tensor("m01t", [N, N], bf, kind="ExternalInput")
    mofft = nc.dram_tensor("mofft", [N, N], f8, kind="ExternalInput")
    wout = nc.dram_tensor("wout", [D, D], bf, kind="ExternalInput")
    emat = nc.dram_tensor("emat", [8, 512], bf, kind="ExternalInput")
    out = nc.dram_tensor("out", [N // 4, D], bf, kind="ExternalOutput")

    with tile.TileContext(nc) as tc:
        with (
            tc.tile_pool(name="res", bufs=1) as res,
            tc.tile_pool(name="dram", bufs=1, space="DRAM") as dram,
            tc.tile_pool(name="pp", bufs=5) as pool_p,
            tc.tile_pool(name="pe", bufs=4) as pool_e,
            tc.tile_pool(name="pao", bufs=3) as pao,
            tc.tile_pool(name="ps_pool", bufs=2, space="PSUM") as ps_pool,
            tc.tile_pool(name="pacc", bufs=2, space="PSUM") as pacc,
        ):
            # ---------------- resident tensors ----------------
            qkt = res.tile([128, 4 * N], bf)      # [qT01|qT23|kT01|kT23]
            v_aug = res.tile([128, NJ * 260], bf)  # per jc: 4x(64 v + ones)
            m01 = res.tile([128, NJ * IB], bf)     # 0/1 mask, current i-half
            moff = res.tile([128, NJ * IB], f8)    # {0,-80} mask, same half
            wout_sb = res.tile([128, 8 * D], bf)
            emat_sb = res.tile([8, 512], bf, name="emat_sb")
            att_sb = [res.tile([128, 4 * 256], bf, name=f"att{r}")
                      for r in range(4)]
            attn_n = [res.tile([128, 4 * 256], bf, name=f"attn{r}")
                      for r in range(4)]
            sums_sb = [res.tile([8, 256], bf, name=f"sums{r}")
                       for r in range(4)]

            ag_ins = [dram.tile([130, IB], bf, name=f"ag_in{r}")
                      for r in range(4)]
            ag_outs = [dram.tile([8 * 130, IB], bf, name=f"ag_out{r}",
                                 addr_space="Shared")
                       for r in range(4)]

            nc.vector.memset(v_aug[:], 1.0)

            pid = nc.sync.partition_id()
            i0c = (pid % 4) * 256    # my 256-col slice within a chunk
            goff = (pid // 4) * 520  # my batch group's rank-block offset

            def load_mask_half(ib2, q):
                for jc in range(NJ):
                    q.dma_start(
                        m01[:, IB * jc:IB * (jc + 1)],
                        m01t[128 * jc:128 * (jc + 1),
                             IB * ib2:IB * (ib2 + 1)])
                    q.dma_start(
                        moff[:, IB * jc:IB * (jc + 1)],
                        mofft[128 * jc:128 * (jc + 1),
                              IB * ib2:IB * (ib2 + 1)])

            with tc.tile_pool(name="ph0", bufs=1) as p0:
                xtr = p0.tile([128, 8 * N], bf)
                wr = p0.tile([128, 8 * 768], bf)

                for k in range(8):
                    nc.gpsimd.dma_start(wr[:, 768 * k:768 * (k + 1)],
                                        wqkv[128 * k:128 * (k + 1), :])
                    nc.gpsimd.dma_start(xtr[:, N * k:N * (k + 1)],
                                        xt[128 * k:128 * (k + 1), :])
                nc.gpsimd.dma_start(emat_sb[:], emat[:])
                load_mask_half(0, nc.gpsimd)
                for k in range(8):
                    nc.gpsimd.dma_start(wout_sb[:, D * k:D * (k + 1)],
                                        wout[128 * k:128 * (k + 1), :])

                # tiny dummy AllGather to warm up the collectives
                # firmware before the first real one (~30us cold start)
                agw_in = dram.tile([8, 16], bf, name="agw_in")
                agw_out = dram.tile([64, 16], bf, name="agw_out",
                                    addr_space="Shared")
                warm_sb = res.tile([8, 16], bf, name="warm_sb")
                nc.vector.memset(warm_sb[:], 0.0)
                nc.sync.dma_start(agw_in[:], warm_sb[:])
                nc.gpsimd.collective_compute(
                    "AllGather",
                    mybir.AluOpType.bypass,
                    replica_groups=[[0, 1, 2, 3, 4, 5, 6, 7]],
                    ins=[agw_in[:].opt()],
                    outs=[agw_out[:].opt()],
                )

                # ---------------- projections (upfront) ----------------
                def proj_qk_group(t_i, nb):
                    wcol = 128 * t_i
                    ps = ps_pool.tile([128, 512], f32, name="ps_qk",
                                      tag="mm")
                    for k in range(8):
                        nc.tensor.matmul(
                            ps[:],
                            wr[:, 768 * k + wcol:768 * k + wcol + 128],
                            xtr[:, N * k + 512 * nb:N * k + 512 * nb + 512],
                            start=(k == 0), stop=(k == 7),
                        )
                    nc.scalar.copy(
                        qkt[:, N * t_i + 512 * nb:N * t_i + 512 * nb + 512],
                        ps[:])

                def proj_v_group(jc):
                    ps = ps_pool.tile([128, 256], f32, name="ps_v", tag="mm")
                    for k in range(8):
                        nc.tensor.matmul(
                            ps[:],
                            xtr[:, N * k + 128 * jc:N * k + 128 * jc + 128],
                            wr[:, 768 * k + 512:768 * k + 768],
                            start=(k == 0), stop=(k == 7),
                        )
                    for h in range(4):
                        nc.vector.tensor_copy(
                            v_aug[:, 260 * jc + 65 * h:260 * jc + 65 * h + 64],
                            ps[:, 64 * h:64 * h + 64])

                for nb in range(4):
                    proj_qk_group(0, nb)
                for nb in range(4):
                    proj_qk_group(2, nb)
                for jc in range(NJ):
                    proj_v_group(jc)
                for nb in range(4):
                    proj_qk_group(1, nb)
                for nb in range(4):
                    proj_qk_group(3, nb)

            # ---------------- merged attention rounds ----------------
            def do_round(r):
                pair, ib2 = ROUNDS[r]
                q_off = N * pair + IB * ib2
                k_off = N * (2 + pair)
                lA, lB = 2 * pair, 2 * pair + 1

                accA = pacc.tile([65, IB], f32, name="accA", tag="acc")
                accB = pacc.tile([65, IB], f32, name="accB", tag="acc")

                def emit_av(jc, pA, pB, bcast):
                    for ih in range(2):
                        nc.tensor.matmul(
                            accA[:, 512 * ih:512 * ih + 512],
                            v_aug[:, 260 * jc + 65 * lA:
                                  260 * jc + 65 * lA + 65],
                            pA[:, 512 * ih:512 * ih + 512],
                            start=(jc == 0), stop=(jc == NJ - 1),
                        )
                    for ih in range(2):
                        rhsB = pB[:, 512 * ih:512 * ih + 512]
                        if bcast:
                            rhsB = rhsB.bitcast(bf)
                        nc.tensor.matmul(
                            accB[:, 512 * ih:512 * ih + 512],
                            v_aug[:, 260 * jc + 65 * lB:
                                  260 * jc + 65 * lB + 65],
                            rhsB,
                            start=(jc == 0), stop=(jc == NJ - 1),
                        )

                # software pipelining: attn@v for jc is emitted 2 slots
                # late so the PE FIFO never blocks scores(jc+1) on jc's
                # exp/mask chain
                pq = []
                for jc in range(NJ):
                    sA = ps_pool.tile([128, IB], f32, name="sA", tag="mm")
                    sB = ps_pool.tile([128, IB], f32, name="sB", tag="mm")
                    for ih in range(2):
                        nc.tensor.matmul(
                            sA[:, 512 * ih:512 * ih + 512],
                            qkt[0:64, k_off + 128 * jc:k_off + 128 * jc + 128],
                            qkt[0:64, q_off + 512 * ih:q_off + 512 * ih + 512],
                            start=True, stop=True, tile_position=(0, 0),
                        )
                        nc.tensor.matmul(
                            sB[:, 512 * ih:512 * ih + 512],
                            qkt[64:128,
                                k_off + 128 * jc:k_off + 128 * jc + 128],
                            qkt[64:128,
                                q_off + 512 * ih:q_off + 512 * ih + 512],
                            start=True, stop=True, tile_position=(64, 0),
                        )
                    mrow = m01[:, IB * jc:IB * jc + IB]
                    morow = moff[:, IB * jc:IB * jc + IB]
                    # head A always: table exp on ACT + 0/1 mask-mul DVE
                    eA = pool_e.tile([128, IB], bf, name="eA", tag="eA")
                    nc.scalar.activation(
                        eA[:], sA[:], mybir.ActivationFunctionType.Exp)
                    if jc % 2 == 0:
                        # head B via ACT exp too; DVE multiplies both
                        eB = pool_e.tile([128, IB], bf, name="eB", tag="eB")
                        nc.scalar.activation(
                            eB[:], sB[:], mybir.ActivationFunctionType.Exp)
                        pA = pool_p.tile([128, IB], bf, name="pA", tag="pA")
                        nc.vector.tensor_mul(pA[:], eA[:], mrow)
                        pB = pool_p.tile([128, IB], bf, name="pB", tag="pB")
                        nc.vector.tensor_mul(pB[:], eB[:], mrow)
                        bcast = False
                    else:
                        # head B: masked Schraudolph - DVE folds the
                        # {0,-80} offset in (ready right after scores,
                        # so emitted before the ACT-gated mulA), GpSimd
                        # does the int16 fma convert
                        aB = pool_e.tile([128, IB], bf, name="aB", tag="aB")
                        nc.vector.tensor_tensor(
                            aB[:], sB[:], morow, mybir.AluOpType.add)
                        pA = pool_p.tile([128, IB], bf, name="pA", tag="pA")
                        nc.vector.tensor_mul(pA[:], eA[:], mrow)
                        pB = pool_p.tile([128, IB], i16, name="tB", tag="pB")
                        nc.gpsimd.tensor_scalar(
                            pB[:], aB[:], A16, B16,
                            mybir.AluOpType.mult, mybir.AluOpType.add)
                        bcast = True
                    pq.append((jc, pA, pB, bcast))
                    if len(pq) > 3:
                        emit_av(*pq.pop(0))
                while pq:
                    emit_av(*pq.pop(0))

                # round tail: evacuate + ship (no normalize here)
                for hh, acc in ((0, accA), (1, accB)):
                    ao = pao.tile([65, IB], bf, name="ao", tag="ao")
                    nc.vector.tensor_copy(ao[:], acc[:])
                    nc.sync.dma_start(
                        ag_ins[r][64 * hh:64 * hh + 64, :], ao[0:64, :])
                    nc.sync.dma_start(
                        ag_ins[r][128 + hh:129 + hh, :], ao[64:65, :])
                nc.gpsimd.collective_compute(
                    "AllGather",
                    mybir.AluOpType.bypass,
                    replica_groups=[[0, 1, 2, 3, 4, 5, 6, 7]],
                    ins=[ag_ins[r][:].opt()],
                    outs=[ag_outs[r][:].opt()],
                )

            def emit_att_reads(r):
                for gp in range(4):
                    nc.sync.dma_start(
                        att_sb[r][:, 256 * gp:256 * gp + 256],
                        ag_outs[r][bass.ds(goff + 130 * gp, 128),
                                   bass.ds(i0c, 256)])
                    nc.sync.dma_start(
                        sums_sb[r][2 * gp:2 * gp + 2, :],
                        ag_outs[r][bass.ds(goff + 130 * gp + 128, 2),
                                   bass.ds(i0c, 256)])

            def normalize(r):
                rec = res.tile([8, 256], bf, name=f"rec{r}")
                with nc.allow_low_precision(reason="softmax recip"):
                    nc.vector.reciprocal(rec[:], sums_sb[r][:])
                for gp in range(4):
                    bc = ps_pool.tile([128, 256], f32, name="bc", tag="mm")
                    nc.tensor.matmul(
                        bc[:], emat_sb[:, 128 * gp:128 * gp + 128], rec[:],
                        start=True, stop=True)
                    with nc.allow_low_precision(reason="softmax norm"):
                        nc.vector.tensor_mul(
                            attn_n[r][:, 256 * gp:256 * gp + 256],
                            att_sb[r][:, 256 * gp:256 * gp + 256],
                            bc[:])

            with tc.tile_pool(name="ph2", bufs=1) as p2:
                do_round(0)
                do_round(1)
                load_mask_half(1, nc.sync)  # refill mask for rounds 3-4
                emit_att_reads(0)   # AG0 fired a round ago
                do_round(2)
                emit_att_reads(1)
                emit_att_reads(2)   # AG2 completes mid-round-4
                do_round(3)

                # ---------------- normalize + output projection ----------------
                def out_proj(ib2):
                    ra = 2 * ib2
                    rb = 2 * ib2 + 1
                    for mo in range(2):
                        for nh in range(2):
                            ps = ps_pool.tile([128, 512], f32, name="ps_o",
                                              tag="mm")
                            ki = 0
                            for gp in range(4):
                                for p, rr in ((0, ra), (1, rb)):
                                    nc.tensor.matmul(
                                        ps[:],
                                        attn_n[rr][:, 256 * gp + 128 * mo:
                                                   256 * gp + 128 * mo + 128],
                                        wout_sb[:, D * (2 * gp + p) + 512 * nh:
                                                D * (2 * gp + p) + 512 * nh + 512],
                                        start=(ki == 0), stop=(ki == 7),
                                    )
                                    ki += 1
                            ot = pao.tile([128, 512], bf, name="ot", tag="ot")
                            nc.scalar.copy(ot[:], ps[:])
                            nc.sync.dma_start(
                                out[256 * ib2 + 128 * mo:
                                    256 * ib2 + 128 * mo + 128,
                                    512 * nh:512 * nh + 512],
                                ot[:])

                normalize(0)
                normalize(1)
                normalize(2)
                out_proj(0)          # overlaps AG3 flight
                emit_att_reads(3)    # waits AG3; nothing queued behind
                normalize(3)
                out_proj(1)

    nc.compile()
    return nc


def _get_nc():
    global _cached_nc
    if _cached_nc is None:
        _cached_nc = _build()
    return _cached_nc


def kernel(x, mask, W_qkv, W_out, b_out):
    x = np.asarray(x, dtype=np.float32)
    mask = np.asarray(mask)
    W_qkv = np.asarray(W_qkv, dtype=np.float32)
    W_out = np.asarray(W_out, dtype=np.float32)
    b_out = np.asarray(b_out, dtype=np.float32)

    nc = _get_nc()

    mt = mask.reshape(N, N).T.astype(np.float32)
    m01_bf = np.ascontiguousarray(mt).astype(BF16)
    moff_f8 = np.ascontiguousarray((mt - 1.0) * (-MOFF)).astype(FP8)
    wout_bf = W_out.astype(BF16)
    # expansion matrix: emat[s, 128g+row] = 1 iff s == 2g + row//64
    em = np.zeros((8, 512), dtype=np.float32)
    for g in range(4):
        for s2 in range(2):
            em[2 * g + s2, 128 * g + 64 * s2:128 * g + 64 * s2 + 64] = 1.0
    em_bf = em.astype(BF16)

    in_maps = []
    for c in range(N_CORES):
        b = c // 4
        g = c % 4
        hs = slice(g * HPC * HD, (g + 1) * HPC * HD)
        wq = W_qkv[:, 0 * D:1 * D][:, hs] * np.float32(SCALE)
        wk = W_qkv[:, 1 * D:2 * D][:, hs]
        wv = W_qkv[:, 2 * D:3 * D][:, hs]
        wqkv_c = np.ascontiguousarray(
            np.concatenate([wq, wk, wv], axis=1)).astype(BF16)
        xt_c = np.ascontiguousarray(x[b].T).astype(BF16)
        in_maps.append({
            "xt": xt_c,
            "wqkv": wqkv_c,
            "m01t": m01_bf,
            "mofft": moff_f8,
            "wout": wout_bf,
            "emat": em_bf,
        })

    global _last_in_maps, _last_res
    _last_in_maps = in_maps

    res = bass_utils.run_bass_kernel_spmd(
        nc, in_maps, core_ids=list(range(N_CORES)))
    _last_res = res

    out_full = np.empty((B, N, D), dtype=np.float32)
    for c in range(N_CORES):
        b = c // 4
        g = c % 4
        core_out = res.results[c]["out"].astype(np.float32)
        out_full[b, 256 * g:256 * g + 256, :] = core_out[0:256]
        out_full[b, 1024 + 256 * g:1024 + 256 * g + 256, :] = core_out[256:512]
    out_full += b_out
    return out_full
